# revision 9
# baseline (speedup 1.0000x reference)
"""Trainium2 Bass kernel for nn_AttentionBlock (B=4, H=W=64, C=512, Cr=64).

Reference computation (per batch sample b):
    xf = x[b].reshape(N=4096, C=512)
    q = xf @ Wf; k = xf @ Wg; v = xf @ Wh
    attn = softmax(q @ k.T, axis=-1)
    out[b] = gamma * (attn @ v) + x[b]

Sharding: 8 cores, data-parallel over B=4 with 2-way sequence-parallel over
query rows. Core c handles batch c//2, query-row half c%2 (2048 rows).
Each core receives the full 4096x512 x of its batch, permuted so its OWN
2048 query rows come first (softmax over keys is permutation invariant as
long as k and v use the same key order, which they do). The program is
identical on all cores (SPMD); only the input data differs.

Per-core dataflow (matmuls bf16, f32 accumulation in PSUM):
  1. DMA x row tiles, cast bf16 on VectorE, PE-transpose -> xT [C, 4096].
  2. qT = Wf.T @ xT[:, :2048]; kT = Wg.T @ xT; v = x @ Wh (per-key-tile).
     kT/qT are duplicated onto partitions 64..127 (SBUF->SBUF DMA) so the
     K=64 score matmuls can row-pack onto disjoint halves of the PE array.
  3. Per 512-row block, key tiles processed in PAIRS: two back-to-back
     score matmuls scoresT[keys,rows] = kT_tile.T @ qT_block on array
     halves h0/h64 run concurrently; one exp over both on ScalarE (no max
     subtraction: |scores| < 60 is fp32/bf16-safe); o += exp.T @ v_tile
     accumulated over all 32 key tiles (software-pipelined one pair deep
     so PE never waits on exp); row sums via N=1 ones-matmuls.
  4. out = (o * gamma/sum) + x fused on VectorE, DMA out.
"""

import sys

if "/opt/trn_rl_repo" not in sys.path:
    sys.path.insert(0, "/opt/trn_rl_repo")

import numpy as np

_BUILt = {}

B, H, W, C = 4, 64, 64, 512
CR = 64          # C // reduction ratio
N = H * W        # 4096 keys per batch
R = N // 2       # 2048 query rows per core
NCORES = 8
NKT = N // 128   # 32 key tiles
KC = C // 128    # 4 contraction chunks over C


def _build():
    import ml_dtypes
    import concourse.bass as bass
    import concourse.mybir as mybir
    import concourse.tile as tile
    from concourse import bacc

    f32 = mybir.dt.float32
    bf16 = mybir.dt.bfloat16
    Exp = mybir.ActivationFunctionType.Exp
    mult = mybir.AluOpType.mult
    add = mybir.AluOpType.add

    nc = bacc.Bacc(
        "TRN2",
        target_bir_lowering=False,
        debug=False,
        num_devices=NCORES,
    )

    x_d = nc.dram_tensor("x", [N, C], f32, kind="ExternalInput")
    wf_d = nc.dram_tensor("wf", [C, CR], f32, kind="ExternalInput")
    wg_d = nc.dram_tensor("wg", [C, CR], f32, kind="ExternalInput")
    wh_d = nc.dram_tensor("wh", [C, C], f32, kind="ExternalInput")
    gam_d = nc.dram_tensor("gammav", [128, 1], f32, kind="ExternalInput")
    out_d = nc.dram_tensor("out", [R, C], f32, kind="ExternalOutput")

    identb_d = nc.inline_tensor(
        np.eye(128, dtype=ml_dtypes.bfloat16), name="identbc"
    )
    ones_d = nc.inline_tensor(
        np.ones((128, 1), dtype=ml_dtypes.bfloat16), name="onesc"
    )

    with tile.TileContext(nc) as tc:
        with (
            tc.tile_pool(name="const", bufs=1) as cpool,
            tc.tile_pool(name="stand", bufs=1) as spool,
            tc.tile_pool(name="xin", bufs=5) as xin_pool,
            tc.tile_pool(name="wtmp", bufs=2) as wtmp_pool,
            tc.tile_pool(name="exp", bufs=3) as ex_pool,
            tc.tile_pool(name="small", bufs=8) as sm_pool,
            tc.tile_pool(name="xres", bufs=4) as xres_pool,
            tc.tile_pool(name="outp", bufs=4) as out_pool,
        ):
            # ---- constants (only the transpose identity up front; the
            # rest rides behind the first x-row DMAs) ----
            identb_sb = cpool.tile([128, 128], bf16, name="identb_sb")
            nc.sync.dma_start(out=identb_sb[:], in_=identb_d[:])
            ones_sb = cpool.tile([128, 1], bf16, name="ones_sb")
            gam_sb = cpool.tile([128, 1], f32, name="gam_sb")

            wf_sb = cpool.tile([128, KC * CR], bf16, name="wf_sb")
            wg_sb = cpool.tile([128, KC * CR], bf16, name="wg_sb")
            wh_sb = cpool.tile([128, KC * C], bf16, name="wh_sb")

            def emit_weights():
                nc.sync.dma_start(out=ones_sb[:], in_=ones_d[:])
                nc.sync.dma_start(out=gam_sb[:], in_=gam_d[:])
                for w_d, w_sb, cols in (
                    (wg_d, wg_sb, CR),
                    (wh_d, wh_sb, C),
                    (wf_d, wf_sb, CR),
                ):
                    for kc in range(KC):
                        wt = wtmp_pool.tile(
                            [128, cols], f32, tag="wt", name="wt"
                        )
                        nc.sync.dma_start(
                            out=wt[:], in_=w_d[kc * 128 : (kc + 1) * 128, :]
                        )
                        nc.vector.tensor_copy(
                            w_sb[:, kc * cols : (kc + 1) * cols], wt[:]
                        )

            # ---- standing bf16 tensors ----
            xTa = [
                spool.tile([128, R], bf16, name=f"xTa{kc}") for kc in range(KC)
            ]
            xTb = [
                spool.tile([128, R], bf16, name=f"xTb{kc}") for kc in range(KC)
            ]
            v_t = [
                spool.tile([128, C], bf16, name=f"v{kt}") for kt in range(NKT)
            ]
            # kT/qT with a duplicate copy on partitions 64..127
            kTd = spool.tile([128, N], bf16, name="kTd")
            qTd = spool.tile([128, R], bf16, name="qTd")

            with (
                tc.tile_pool(name="ps12", bufs=3, space="PSUM") as tp_pool,
                tc.tile_pool(name="ps2kq", bufs=2, space="PSUM") as kq_pool,
                tc.tile_pool(name="ps2v", bufs=2, space="PSUM") as vps_pool,
            ):
                # ---- phase 1+2: transpose x; compute qT, kT, v ----
                def load_transpose_half(xT, half):
                    for rt16 in range(16):
                        rt = half * 16 + rt16
                        if rt == 8:
                            emit_weights()
                        xt = xin_pool.tile([128, C], f32, tag="xt", name="xt")
                        nc.sync.dma_start(
                            out=xt[:], in_=x_d[rt * 128 : (rt + 1) * 128, :]
                        )
                        xb = xin_pool.tile([128, C], bf16, tag="xb", name="xb")
                        # split casts DVE/GpSimd so the PE transposes don't
                        # wait on a serial DVE cast chain
                        if rt16 % 4 == 3:
                            nc.gpsimd.tensor_copy(xb[:], xt[:])
                        else:
                            nc.vector.tensor_copy(xb[:], xt[:])
                        tpt = tp_pool.tile(
                            [128, 512], bf16, tag="tp", name="tpt"
                        )
                        for kc in range(KC):
                            nc.tensor.transpose(
                                tpt[:, kc * 128 : (kc + 1) * 128],
                                xb[:, kc * 128 : (kc + 1) * 128],
                                identb_sb[:],
                            )
                        for kc in range(KC):
                            dst = xT[kc][:, rt16 * 128 : (rt16 + 1) * 128]
                            src = tpt[:, kc * 128 : (kc + 1) * 128]
                            if kc % 2 == 0:
                                nc.vector.tensor_copy(dst, src)
                            else:
                                nc.scalar.copy(dst, src)

                def emit_kq(w_sb, dst_sb, xT, nt_local, dst_off):
                    ps = kq_pool.tile([CR, 512], f32, tag="kq", name="kqp")
                    for kc in range(KC):
                        nc.tensor.matmul(
                            ps[:],
                            lhsT=w_sb[:, kc * CR : (kc + 1) * CR],
                            rhs=xT[kc][:, nt_local * 512 : (nt_local + 1) * 512],
                            start=(kc == 0),
                            stop=(kc == KC - 1),
                        )
                    nc.scalar.copy(dst_sb[0:CR, dst_off : dst_off + 512], ps[:])

                def emit_v(xT, kt):
                    kt16 = kt % 16
                    ps = vps_pool.tile([128, C], f32, tag="vps", name="vp")
                    for kc in range(KC):
                        nc.tensor.matmul(
                            ps[:],
                            lhsT=xT[kc][:, kt16 * 128 : (kt16 + 1) * 128],
                            rhs=wh_sb[:, kc * C : (kc + 1) * C],
                            start=(kc == 0),
                            stop=(kc == KC - 1),
                        )
                    nc.scalar.copy(v_t[kt][:], ps[:])

                load_transpose_half(xTa, 0)
                for nt in range(R // 512):
                    emit_kq(wf_sb, qTd, xTa, nt, nt * 512)
                # duplicate qT onto partitions 64..127
                nc.sync.dma_start(out=qTd[CR:128, :], in_=qTd[0:CR, :])
                for nt in range(4):
                    emit_kq(wg_sb, kTd, xTa, nt, nt * 512)
                for kt in range(16):
                    emit_v(xTa, kt)
                load_transpose_half(xTb, 1)
                for nt in range(4):
                    emit_kq(wg_sb, kTd, xTb, nt, 2048 + nt * 512)
                for kt in range(16, 32):
                    emit_v(xTb, kt)
                # duplicate kT onto partitions 64..127
                nc.sync.dma_start(out=kTd[CR:128, :], in_=kTd[0:CR, :])

            # ---- phase 3: attention over 4 blocks of 512 query rows ----
            # Key tiles processed in pairs: two row-packed score matmuls,
            # one exp, then (pipelined one pair back) 8 o-matmuls + sums.
            with tc.tile_pool(name="ps3", bufs=1, space="PSUM") as p3:
                # prefetch all residual x row tiles (DMA is idle in phase 3)
                xr_tiles = []
                for rt in range(16):
                    xr = xres_pool.tile(
                        [128, C], f32, tag=f"xr{rt}", bufs=1, name=f"xr{rt}"
                    )
                    nc.sync.dma_start(
                        out=xr[:], in_=x_d[rt * 128 : (rt + 1) * 128, :]
                    )
                    xr_tiles.append(xr)
                for blk in range(4):
                    o_ps = [
                        p3.tile(
                            [128, C], f32, tag=f"o{rc}", name=f"ops{blk}_{rc}"
                        )
                        for rc in range(4)
                    ]
                    s_ps = p3.tile([128, 4, 2], bf16, tag="sums", name=f"sps{blk}")
                    srow = p3.tile([1, 512], f32, tag="srow", name=f"srow{blk}")

                    def emit_o_pair(expair, ktbase):
                        # add the pair's two key tiles on VectorE; row sums
                        # come from ONE swapped-operand matmul per pair
                        # (ones as the 1-col stationary -> cheap LDWEIGHTS),
                        # accumulated in a [1, 512] psum row
                        exs = ex_pool.tile(
                            [128, 512], bf16, tag="exs", bufs=2, name="exs"
                        )
                        nc.vector.tensor_add(
                            exs[:], expair[:, 0:512], expair[:, 512:1024]
                        )
                        for sub in range(2):
                            kt = ktbase + sub
                            for rc in range(4):
                                lhs = expair[
                                    :,
                                    sub * 512
                                    + rc * 128 : sub * 512
                                    + (rc + 1) * 128,
                                ]
                                nc.tensor.matmul(
                                    o_ps[rc][:],
                                    lhsT=lhs,
                                    rhs=v_t[kt][:],
                                    start=(kt == 0),
                                    stop=(kt == NKT - 1),
                                )
                        nc.tensor.matmul(
                            srow[:],
                            lhsT=ones_sb[:],
                            rhs=exs[:],
                            start=(ktbase == 0),
                            stop=(ktbase == NKT - 2),
                        )

                    prev = None
                    for ktp in range(NKT // 2):
                        scp = p3.tile(
                            [128, 1024], f32, tag="sc", bufs=1, name="scp"
                        )
                        for sub in range(2):
                            kt = 2 * ktp + sub
                            hp = sub * CR
                            nc.tensor.matmul(
                                scp[:, sub * 512 : (sub + 1) * 512],
                                lhsT=kTd[
                                    hp : hp + CR, kt * 128 : (kt + 1) * 128
                                ],
                                rhs=qTd[
                                    hp : hp + CR,
                                    blk * 512 : (blk + 1) * 512,
                                ],
                                start=True,
                                stop=True,
                            )
                        expair = ex_pool.tile(
                            [128, 1024], bf16, tag="ex", name="ex"
                        )
                        nc.scalar.activation(expair[:], scp[:], Exp)
                        if prev is not None:
                            emit_o_pair(*prev)
                        prev = (expair, 2 * ktp)
                    emit_o_pair(*prev)

                    # bring the [1, 512] sum row back to [128, 4] layout via
                    # four tiny PE transposes (sbuf staging copy first)
                    ssb = sm_pool.tile([1, 512], bf16, tag="ssb", name="ssb")
                    nc.vector.tensor_copy(ssb[:], srow[:])
                    for rc in range(4):
                        nc.tensor.transpose(
                            s_ps[:, rc, 0:1],
                            ssb[0:1, rc * 128 : (rc + 1) * 128],
                            identb_sb[0:1, 0:1],
                        )
                    scls = []
                    for rc in range(4):
                        rcp = sm_pool.tile([128, 1], f32, tag="rcp", name="rcp")
                        nc.vector.reciprocal(rcp[:], s_ps[:, rc, 0:1])
                        scl = sm_pool.tile([128, 1], f32, tag="scl", name="scl")
                        nc.vector.tensor_scalar_mul(scl[:], rcp[:], gam_sb[:])
                        scls.append(scl)
                    for rc in range(4):
                        rt = blk * 4 + rc
                        ot = out_pool.tile([128, C], f32, tag="ot", name="ot")
                        nc.vector.scalar_tensor_tensor(
                            out=ot[:],
                            in0=o_ps[rc][:],
                            scalar=scls[rc][:],
                            in1=xr_tiles[rt][:],
                            op0=mult,
                            op1=add,
                        )
                        nc.sync.dma_start(
                            out=out_d[rt * 128 : (rt + 1) * 128, :], in_=ot[:]
                        )

    nc.compile()
    return nc


def _get_nc():
    if "nc" not in _BUILt:
        _BUILt["nc"] = _build()
    return _BUILt["nc"]


def make_in_maps(x, Wf, Wg, Wh, gamma):
    x = np.asarray(x, dtype=np.float32)
    gv = np.full((128, 1), np.float32(np.asarray(gamma).reshape(-1)[0]))
    wf = np.ascontiguousarray(np.asarray(Wf, dtype=np.float32))
    wg = np.ascontiguousarray(np.asarray(Wg, dtype=np.float32))
    wh = np.ascontiguousarray(np.asarray(Wh, dtype=np.float32))
    in_maps = []
    for core in range(NCORES):
        b, h = divmod(core, 2)
        xb = x[b].reshape(N, C)
        own = xb[h * R : (h + 1) * R]
        other = xb[(1 - h) * R : (2 - h) * R]
        xp = np.ascontiguousarray(np.concatenate([own, other], axis=0))
        in_maps.append(
            {"x": xp, "wf": wf, "wg": wg, "wh": wh, "gammav": gv}
        )
    return in_maps


def gather_out(results, x):
    out = np.empty((B, N, C), dtype=np.float32)
    for core in range(NCORES):
        b, h = divmod(core, 2)
        out[b, h * R : (h + 1) * R] = results[core]["out"]
    return out.reshape(B, H, W, C)


def run(x, Wf, Wg, Wh, gamma, **spmd_kwargs):
    from concourse.bass_utils import run_bass_kernel_spmd

    nc = _get_nc()
    in_maps = make_in_maps(x, Wf, Wg, Wh, gamma)
    res = run_bass_kernel_spmd(
        nc, in_maps, core_ids=list(range(NCORES)), **spmd_kwargs
    )
    return gather_out(res.results, x), res


def kernel(x, Wf, Wg, Wh, gamma):
    out, _ = run(x, Wf, Wg, Wh, gamma)
    return out



# revision 10
# speedup vs baseline: 1.0779x; 1.0779x over previous
"""Trainium2 Bass kernel for nn_AttentionBlock (B=4, H=W=64, C=512, Cr=64).

Reference computation (per batch sample b):
    xf = x[b].reshape(N=4096, C=512)
    q = xf @ Wf; k = xf @ Wg; v = xf @ Wh
    attn = softmax(q @ k.T, axis=-1)
    out[b] = gamma * (attn @ v) + x[b]

Sharding: 8 cores, data-parallel over B=4 with 2-way sequence-parallel over
query rows. Core c handles batch c//2, query-row half c%2 (2048 rows).
Each core receives the full 4096x512 x of its batch, permuted so its OWN
2048 query rows come first (softmax over keys is permutation invariant as
long as k and v use the same key order, which they do). The program is
identical on all cores (SPMD); only the input data differs.

Per-core dataflow (matmuls bf16, f32 accumulation in PSUM):
  1. DMA x row tiles, cast bf16 on VectorE, PE-transpose -> xT [C, 4096].
  2. qT = Wf.T @ xT[:, :2048]; kT = Wg.T @ xT; v = x @ Wh (per-key-tile).
     kT/qT are duplicated onto partitions 64..127 (SBUF->SBUF DMA) so the
     K=64 score matmuls can row-pack onto disjoint halves of the PE array.
  3. Per 512-row block, key tiles processed in PAIRS: two back-to-back
     score matmuls scoresT[keys,rows] = kT_tile.T @ qT_block on array
     halves h0/h64 run concurrently; one exp over both on ScalarE (no max
     subtraction: |scores| < 60 is fp32/bf16-safe); o += exp.T @ v_tile
     accumulated over all 32 key tiles (software-pipelined one pair deep
     so PE never waits on exp); row sums via N=1 ones-matmuls.
  4. out = (o * gamma/sum) + x fused on VectorE, DMA out.
"""

import sys

if "/opt/trn_rl_repo" not in sys.path:
    sys.path.insert(0, "/opt/trn_rl_repo")

import numpy as np

_BUILt = {}

B, H, W, C = 4, 64, 64, 512
CR = 64          # C // reduction ratio
N = H * W        # 4096 keys per batch
R = N // 2       # 2048 query rows per core
NCORES = 8
NKT = N // 128   # 32 key tiles
KC = C // 128    # 4 contraction chunks over C


def _build():
    import ml_dtypes
    import concourse.bass as bass
    import concourse.mybir as mybir
    import concourse.tile as tile
    from concourse import bacc

    f32 = mybir.dt.float32
    bf16 = mybir.dt.bfloat16
    Exp = mybir.ActivationFunctionType.Exp
    mult = mybir.AluOpType.mult
    add = mybir.AluOpType.add

    nc = bacc.Bacc(
        "TRN2",
        target_bir_lowering=False,
        debug=False,
        num_devices=NCORES,
    )

    x_d = nc.dram_tensor("x", [N, C], f32, kind="ExternalInput")
    wf_d = nc.dram_tensor("wf", [C, CR], f32, kind="ExternalInput")
    wg_d = nc.dram_tensor("wg", [C, CR], f32, kind="ExternalInput")
    wh_d = nc.dram_tensor("wh", [C, C], f32, kind="ExternalInput")
    gam_d = nc.dram_tensor("gammav", [128, 1], f32, kind="ExternalInput")
    out_d = nc.dram_tensor("out", [R, C], f32, kind="ExternalOutput")

    identb_d = nc.inline_tensor(
        np.eye(128, dtype=ml_dtypes.bfloat16), name="identbc"
    )
    ones_d = nc.inline_tensor(
        np.ones((128, 1), dtype=ml_dtypes.bfloat16), name="onesc"
    )

    with tile.TileContext(nc) as tc:
        with (
            tc.tile_pool(name="const", bufs=1) as cpool,
            tc.tile_pool(name="stand", bufs=1) as spool,
            tc.tile_pool(name="xin", bufs=8) as xin_pool,
            tc.tile_pool(name="wtmp", bufs=2) as wtmp_pool,
            tc.tile_pool(name="exp", bufs=3) as ex_pool,
            tc.tile_pool(name="small", bufs=8) as sm_pool,
            tc.tile_pool(name="xres", bufs=4) as xres_pool,
            tc.tile_pool(name="outp", bufs=4) as out_pool,
        ):
            # ---- constants (only the transpose identity up front; the
            # rest rides behind the first x-row DMAs) ----
            identb_sb = cpool.tile([128, 128], bf16, name="identb_sb")
            nc.sync.dma_start(out=identb_sb[:], in_=identb_d[:])
            ones_sb = cpool.tile([128, 1], bf16, name="ones_sb")
            gam_sb = cpool.tile([128, 1], f32, name="gam_sb")

            wf_sb = cpool.tile([128, KC * CR], bf16, name="wf_sb")
            wg_sb = cpool.tile([128, KC * CR], bf16, name="wg_sb")
            wh_sb = cpool.tile([128, KC * C], bf16, name="wh_sb")

            def emit_weights():
                nc.sync.dma_start(out=ones_sb[:], in_=ones_d[:])
                nc.sync.dma_start(out=gam_sb[:], in_=gam_d[:])
                for w_d, w_sb, cols in (
                    (wg_d, wg_sb, CR),
                    (wh_d, wh_sb, C),
                    (wf_d, wf_sb, CR),
                ):
                    for kc in range(KC):
                        wt = wtmp_pool.tile(
                            [128, cols], f32, tag="wt", name="wt"
                        )
                        nc.sync.dma_start(
                            out=wt[:], in_=w_d[kc * 128 : (kc + 1) * 128, :]
                        )
                        nc.vector.tensor_copy(
                            w_sb[:, kc * cols : (kc + 1) * cols], wt[:]
                        )

            # ---- standing bf16 tensors ----
            xTa = [
                spool.tile([128, R], bf16, name=f"xTa{kc}") for kc in range(KC)
            ]
            xTb = [
                spool.tile([128, R], bf16, name=f"xTb{kc}") for kc in range(KC)
            ]
            v_t = [
                spool.tile([128, C], bf16, name=f"v{kt}") for kt in range(NKT)
            ]
            # kT/qT with a duplicate copy on partitions 64..127
            kTd = spool.tile([128, N], bf16, name="kTd")
            qTd = spool.tile([128, R], bf16, name="qTd")

            with (
                tc.tile_pool(name="ps12", bufs=3, space="PSUM") as tp_pool,
                tc.tile_pool(name="ps2kq", bufs=2, space="PSUM") as kq_pool,
                tc.tile_pool(name="ps2v", bufs=2, space="PSUM") as vps_pool,
            ):
                # ---- phase 1+2: transpose x; compute qT, kT, v ----
                def load_transpose_half(xT, half):
                    for rt16 in range(16):
                        rt = half * 16 + rt16
                        if rt == 8:
                            emit_weights()
                        xt = xin_pool.tile([128, C], f32, tag="xt", name="xt")
                        nc.sync.dma_start(
                            out=xt[:], in_=x_d[rt * 128 : (rt + 1) * 128, :]
                        )
                        xb = xin_pool.tile([128, C], bf16, tag="xb", name="xb")
                        # split casts DVE/GpSimd so PE transposes don't wait
                        # on a serial DVE cast chain
                        if rt16 % 4 == 3:
                            nc.gpsimd.tensor_copy(xb[:], xt[:])
                        else:
                            nc.vector.tensor_copy(xb[:], xt[:])
                        tpt = tp_pool.tile(
                            [128, 512], bf16, tag="tp", name="tpt"
                        )
                        for kc in range(KC):
                            nc.tensor.transpose(
                                tpt[:, kc * 128 : (kc + 1) * 128],
                                xb[:, kc * 128 : (kc + 1) * 128],
                                identb_sb[:],
                            )
                        for kc in range(KC):
                            dst = xT[kc][:, rt16 * 128 : (rt16 + 1) * 128]
                            src = tpt[:, kc * 128 : (kc + 1) * 128]
                            if kc % 2 == 0:
                                nc.vector.tensor_copy(dst, src)
                            else:
                                nc.scalar.copy(dst, src)

                def emit_kq(w_sb, dst_sb, xT, nt_local, dst_off):
                    ps = kq_pool.tile([CR, 512], f32, tag="kq", name="kqp")
                    for kc in range(KC):
                        nc.tensor.matmul(
                            ps[:],
                            lhsT=w_sb[:, kc * CR : (kc + 1) * CR],
                            rhs=xT[kc][:, nt_local * 512 : (nt_local + 1) * 512],
                            start=(kc == 0),
                            stop=(kc == KC - 1),
                        )
                    nc.scalar.copy(dst_sb[0:CR, dst_off : dst_off + 512], ps[:])

                def emit_v(xT, kt):
                    kt16 = kt % 16
                    ps = vps_pool.tile([128, C], f32, tag="vps", name="vp")
                    for kc in range(KC):
                        nc.tensor.matmul(
                            ps[:],
                            lhsT=xT[kc][:, kt16 * 128 : (kt16 + 1) * 128],
                            rhs=wh_sb[:, kc * C : (kc + 1) * C],
                            start=(kc == 0),
                            stop=(kc == KC - 1),
                        )
                    nc.scalar.copy(v_t[kt][:], ps[:])

                load_transpose_half(xTa, 0)
                for nt in range(R // 512):
                    emit_kq(wf_sb, qTd, xTa, nt, nt * 512)
                # duplicate qT onto partitions 64..127
                nc.sync.dma_start(out=qTd[CR:128, :], in_=qTd[0:CR, :])
                for nt in range(4):
                    emit_kq(wg_sb, kTd, xTa, nt, nt * 512)
                for kt in range(16):
                    emit_v(xTa, kt)
                load_transpose_half(xTb, 1)
                for nt in range(4):
                    emit_kq(wg_sb, kTd, xTb, nt, 2048 + nt * 512)
                for kt in range(16, 32):
                    emit_v(xTb, kt)
                # duplicate kT onto partitions 64..127
                nc.sync.dma_start(out=kTd[CR:128, :], in_=kTd[0:CR, :])

            # ---- phase 3: attention over 4 blocks of 512 query rows ----
            # Key tiles processed in pairs: two row-packed score matmuls,
            # one exp, then (pipelined one pair back) 8 o-matmuls + sums.
            with tc.tile_pool(name="ps3", bufs=1, space="PSUM") as p3:
                # prefetch all residual x row tiles (DMA is idle in phase 3)
                xr_tiles = []
                for rt in range(16):
                    xr = xres_pool.tile(
                        [128, C], f32, tag=f"xr{rt}", bufs=1, name=f"xr{rt}"
                    )
                    nc.sync.dma_start(
                        out=xr[:], in_=x_d[rt * 128 : (rt + 1) * 128, :]
                    )
                    xr_tiles.append(xr)
                for blk in range(4):
                    o_ps = [
                        p3.tile(
                            [128, C], f32, tag=f"o{rc}", name=f"ops{blk}_{rc}"
                        )
                        for rc in range(4)
                    ]
                    s_ps = p3.tile([128, 4], f32, tag="sums", name=f"sps{blk}")

                    def emit_o_pair(expair, ktbase):
                        # add the pair's two key tiles on VectorE so the
                        # row-sum needs only 4 ones-matmuls per pair, not 8
                        exs = ex_pool.tile(
                            [128, 512], bf16, tag="exs", bufs=2, name="exs"
                        )
                        nc.vector.tensor_add(
                            exs[:], expair[:, 0:512], expair[:, 512:1024]
                        )
                        for sub in range(2):
                            kt = ktbase + sub
                            for rc in range(4):
                                lhs = expair[
                                    :,
                                    sub * 512
                                    + rc * 128 : sub * 512
                                    + (rc + 1) * 128,
                                ]
                                nc.tensor.matmul(
                                    o_ps[rc][:],
                                    lhsT=lhs,
                                    rhs=v_t[kt][:],
                                    start=(kt == 0),
                                    stop=(kt == NKT - 1),
                                )
                                if sub == 1:
                                    nc.tensor.matmul(
                                        s_ps[:, rc : rc + 1],
                                        lhsT=exs[:, rc * 128 : (rc + 1) * 128],
                                        rhs=ones_sb[:],
                                        start=(ktbase == 0),
                                        stop=(ktbase == NKT - 2),
                                        skip_group_check=True,
                                    )

                    prev = None
                    for ktp in range(NKT // 2):
                        scp = p3.tile(
                            [128, 1024], f32, tag="sc", bufs=1, name="scp"
                        )
                        for sub in range(2):
                            kt = 2 * ktp + sub
                            hp = sub * CR
                            nc.tensor.matmul(
                                scp[:, sub * 512 : (sub + 1) * 512],
                                lhsT=kTd[
                                    hp : hp + CR, kt * 128 : (kt + 1) * 128
                                ],
                                rhs=qTd[
                                    hp : hp + CR,
                                    blk * 512 : (blk + 1) * 512,
                                ],
                                start=True,
                                stop=True,
                            )
                        expair = ex_pool.tile(
                            [128, 1024], bf16, tag="ex", name="ex"
                        )
                        nc.scalar.activation(expair[:], scp[:], Exp)
                        if prev is not None:
                            emit_o_pair(*prev)
                        prev = (expair, 2 * ktp)
                    emit_o_pair(*prev)

                    scls = []
                    for rc in range(4):
                        rcp = sm_pool.tile([128, 1], f32, tag="rcp", name="rcp")
                        nc.vector.reciprocal(rcp[:], s_ps[:, rc : rc + 1])
                        scl = sm_pool.tile([128, 1], f32, tag="scl", name="scl")
                        nc.vector.tensor_scalar_mul(scl[:], rcp[:], gam_sb[:])
                        scls.append(scl)
                    for rc in range(4):
                        rt = blk * 4 + rc
                        ot = out_pool.tile([128, C], f32, tag="ot", name="ot")
                        nc.vector.scalar_tensor_tensor(
                            out=ot[:],
                            in0=o_ps[rc][:],
                            scalar=scls[rc][:],
                            in1=xr_tiles[rt][:],
                            op0=mult,
                            op1=add,
                        )
                        nc.sync.dma_start(
                            out=out_d[rt * 128 : (rt + 1) * 128, :], in_=ot[:]
                        )

    nc.compile()
    return nc


def _get_nc():
    if "nc" not in _BUILt:
        _BUILt["nc"] = _build()
    return _BUILt["nc"]


def make_in_maps(x, Wf, Wg, Wh, gamma):
    x = np.asarray(x, dtype=np.float32)
    gv = np.full((128, 1), np.float32(np.asarray(gamma).reshape(-1)[0]))
    wf = np.ascontiguousarray(np.asarray(Wf, dtype=np.float32))
    wg = np.ascontiguousarray(np.asarray(Wg, dtype=np.float32))
    wh = np.ascontiguousarray(np.asarray(Wh, dtype=np.float32))
    in_maps = []
    for core in range(NCORES):
        b, h = divmod(core, 2)
        xb = x[b].reshape(N, C)
        own = xb[h * R : (h + 1) * R]
        other = xb[(1 - h) * R : (2 - h) * R]
        xp = np.ascontiguousarray(np.concatenate([own, other], axis=0))
        in_maps.append(
            {"x": xp, "wf": wf, "wg": wg, "wh": wh, "gammav": gv}
        )
    return in_maps


def gather_out(results, x):
    out = np.empty((B, N, C), dtype=np.float32)
    for core in range(NCORES):
        b, h = divmod(core, 2)
        out[b, h * R : (h + 1) * R] = results[core]["out"]
    return out.reshape(B, H, W, C)


def run(x, Wf, Wg, Wh, gamma, **spmd_kwargs):
    from concourse.bass_utils import run_bass_kernel_spmd

    nc = _get_nc()
    in_maps = make_in_maps(x, Wf, Wg, Wh, gamma)
    res = run_bass_kernel_spmd(
        nc, in_maps, core_ids=list(range(NCORES)), **spmd_kwargs
    )
    return gather_out(res.results, x), res


def kernel(x, Wf, Wg, Wh, gamma):
    out, _ = run(x, Wf, Wg, Wh, gamma)
    return out



# revision 11
# speedup vs baseline: 1.1031x; 1.0233x over previous
"""Trainium2 Bass kernel for nn_AttentionBlock (B=4, H=W=64, C=512, Cr=64).

Reference computation (per batch sample b):
    xf = x[b].reshape(N=4096, C=512)
    q = xf @ Wf; k = xf @ Wg; v = xf @ Wh
    attn = softmax(q @ k.T, axis=-1)
    out[b] = gamma * (attn @ v) + x[b]

Sharding: 8 cores, data-parallel over B=4 with 2-way sequence-parallel over
query rows. Core c handles batch c//2, query-row half c%2 (2048 rows).
Each core receives the full 4096x512 x of its batch, permuted so its OWN
2048 query rows come first (softmax over keys is permutation invariant as
long as k and v use the same key order, which they do). The program is
identical on all cores (SPMD); only the input data differs.

Per-core dataflow (matmuls bf16, f32 accumulation in PSUM):
  1. DMA x row tiles, cast bf16 on VectorE, PE-transpose -> xT [C, 4096].
  2. qT = Wf.T @ xT[:, :2048]; kT = Wg.T @ xT; v = x @ Wh (per-key-tile).
     kT/qT are duplicated onto partitions 64..127 (SBUF->SBUF DMA) so the
     K=64 score matmuls can row-pack onto disjoint halves of the PE array.
  3. Per 512-row block, key tiles processed in PAIRS: two back-to-back
     score matmuls scoresT[keys,rows] = kT_tile.T @ qT_block on array
     halves h0/h64 run concurrently; one exp over both on ScalarE (no max
     subtraction: |scores| < 60 is fp32/bf16-safe); o += exp.T @ v_tile
     accumulated over all 32 key tiles (software-pipelined one pair deep
     so PE never waits on exp); row sums via N=1 ones-matmuls.
  4. out = (o * gamma/sum) + x fused on VectorE, DMA out.
"""

import sys

if "/opt/trn_rl_repo" not in sys.path:
    sys.path.insert(0, "/opt/trn_rl_repo")

import numpy as np

_BUILt = {}

B, H, W, C = 4, 64, 64, 512
CR = 64          # C // reduction ratio
N = H * W        # 4096 keys per batch
R = N // 2       # 2048 query rows per core
NCORES = 8
NKT = N // 128   # 32 key tiles
KC = C // 128    # 4 contraction chunks over C


def _build():
    import ml_dtypes
    import concourse.bass as bass
    import concourse.mybir as mybir
    import concourse.tile as tile
    from concourse import bacc

    f32 = mybir.dt.float32
    bf16 = mybir.dt.bfloat16
    Exp = mybir.ActivationFunctionType.Exp
    mult = mybir.AluOpType.mult
    add = mybir.AluOpType.add

    nc = bacc.Bacc(
        "TRN2",
        target_bir_lowering=False,
        debug=False,
        num_devices=NCORES,
    )

    x_d = nc.dram_tensor("x", [N, C], f32, kind="ExternalInput")
    wf_d = nc.dram_tensor("wf", [C, CR], f32, kind="ExternalInput")
    wg_d = nc.dram_tensor("wg", [C, CR], f32, kind="ExternalInput")
    wh_d = nc.dram_tensor("wh", [C, C], f32, kind="ExternalInput")
    gam_d = nc.dram_tensor("gammav", [128, 1], f32, kind="ExternalInput")
    out_d = nc.dram_tensor("out", [R, C], f32, kind="ExternalOutput")

    identb_d = nc.inline_tensor(
        np.eye(128, dtype=ml_dtypes.bfloat16), name="identbc"
    )
    ones_d = nc.inline_tensor(
        np.ones((128, 1), dtype=ml_dtypes.bfloat16), name="onesc"
    )

    with tile.TileContext(nc) as tc:
        with (
            tc.tile_pool(name="const", bufs=1) as cpool,
            tc.tile_pool(name="stand", bufs=1) as spool,
            tc.tile_pool(name="xin", bufs=5) as xin_pool,
            tc.tile_pool(name="wtmp", bufs=2) as wtmp_pool,
            tc.tile_pool(name="exp", bufs=3) as ex_pool,
            tc.tile_pool(name="small", bufs=8) as sm_pool,
            tc.tile_pool(name="xres", bufs=4) as xres_pool,
            tc.tile_pool(name="outp", bufs=4) as out_pool,
        ):
            # ---- constants (only the transpose identity up front; the
            # rest rides behind the first x-row DMAs) ----
            identb_sb = cpool.tile([128, 128], bf16, name="identb_sb")
            nc.sync.dma_start(out=identb_sb[:], in_=identb_d[:])
            ones_sb = cpool.tile([128, 1], bf16, name="ones_sb")
            gam_sb = cpool.tile([128, 1], f32, name="gam_sb")

            wf_sb = cpool.tile([128, KC * CR], bf16, name="wf_sb")
            wg_sb = cpool.tile([128, KC * CR], bf16, name="wg_sb")
            wh_sb = cpool.tile([128, KC * C], bf16, name="wh_sb")

            def emit_weights():
                nc.sync.dma_start(out=ones_sb[:], in_=ones_d[:])
                nc.sync.dma_start(out=gam_sb[:], in_=gam_d[:])
                for w_d, w_sb, cols in (
                    (wg_d, wg_sb, CR),
                    (wh_d, wh_sb, C),
                    (wf_d, wf_sb, CR),
                ):
                    for kc in range(KC):
                        wt = wtmp_pool.tile(
                            [128, cols], f32, tag="wt", name="wt"
                        )
                        nc.sync.dma_start(
                            out=wt[:], in_=w_d[kc * 128 : (kc + 1) * 128, :]
                        )
                        nc.vector.tensor_copy(
                            w_sb[:, kc * cols : (kc + 1) * cols], wt[:]
                        )

            # ---- standing bf16 tensors ----
            xTa = [
                spool.tile([128, R], bf16, name=f"xTa{kc}") for kc in range(KC)
            ]
            xTb = [
                spool.tile([128, R], bf16, name=f"xTb{kc}") for kc in range(KC)
            ]
            v_t = [
                spool.tile([128, C], bf16, name=f"v{kt}") for kt in range(NKT)
            ]
            # kT/qT with a duplicate copy on partitions 64..127
            kTd = spool.tile([128, N], bf16, name="kTd")
            qTd = spool.tile([128, R], bf16, name="qTd")

            with (
                tc.tile_pool(name="ps12", bufs=3, space="PSUM") as tp_pool,
                tc.tile_pool(name="ps2kq", bufs=2, space="PSUM") as kq_pool,
                tc.tile_pool(name="ps2v", bufs=2, space="PSUM") as vps_pool,
            ):
                # ---- phase 1+2: transpose x; compute qT, kT, v ----
                def load_transpose_half(xT, half):
                    for rt16 in range(16):
                        rt = half * 16 + rt16
                        if rt == 8:
                            emit_weights()
                        xt = xin_pool.tile([128, C], f32, tag="xt", name="xt")
                        nc.sync.dma_start(
                            out=xt[:], in_=x_d[rt * 128 : (rt + 1) * 128, :]
                        )
                        xb = xin_pool.tile([128, C], bf16, tag="xb", name="xb")
                        nc.vector.tensor_copy(xb[:], xt[:])
                        tpt = tp_pool.tile(
                            [128, 512], bf16, tag="tp", name="tpt"
                        )
                        for kc in range(KC):
                            nc.tensor.transpose(
                                tpt[:, kc * 128 : (kc + 1) * 128],
                                xb[:, kc * 128 : (kc + 1) * 128],
                                identb_sb[:],
                            )
                        for kc in range(KC):
                            dst = xT[kc][:, rt16 * 128 : (rt16 + 1) * 128]
                            src = tpt[:, kc * 128 : (kc + 1) * 128]
                            if kc % 2 == 0:
                                nc.vector.tensor_copy(dst, src)
                            else:
                                nc.scalar.copy(dst, src)

                def emit_kq(w_sb, dst_sb, xT, nt_local, dst_off):
                    ps = kq_pool.tile([CR, 512], f32, tag="kq", name="kqp")
                    for kc in range(KC):
                        nc.tensor.matmul(
                            ps[:],
                            lhsT=w_sb[:, kc * CR : (kc + 1) * CR],
                            rhs=xT[kc][:, nt_local * 512 : (nt_local + 1) * 512],
                            start=(kc == 0),
                            stop=(kc == KC - 1),
                        )
                    nc.scalar.copy(dst_sb[0:CR, dst_off : dst_off + 512], ps[:])

                def emit_v(xT, kt):
                    kt16 = kt % 16
                    ps = vps_pool.tile([128, C], f32, tag="vps", name="vp")
                    for kc in range(KC):
                        nc.tensor.matmul(
                            ps[:],
                            lhsT=xT[kc][:, kt16 * 128 : (kt16 + 1) * 128],
                            rhs=wh_sb[:, kc * C : (kc + 1) * C],
                            start=(kc == 0),
                            stop=(kc == KC - 1),
                        )
                    nc.scalar.copy(v_t[kt][:], ps[:])

                load_transpose_half(xTa, 0)
                for nt in range(R // 512):
                    emit_kq(wf_sb, qTd, xTa, nt, nt * 512)
                # duplicate qT onto partitions 64..127
                nc.sync.dma_start(out=qTd[CR:128, :], in_=qTd[0:CR, :])
                for nt in range(4):
                    emit_kq(wg_sb, kTd, xTa, nt, nt * 512)
                for kt in range(16):
                    emit_v(xTa, kt)
                load_transpose_half(xTb, 1)
                for nt in range(4):
                    emit_kq(wg_sb, kTd, xTb, nt, 2048 + nt * 512)
                for kt in range(16, 32):
                    emit_v(xTb, kt)
                # duplicate kT onto partitions 64..127
                nc.sync.dma_start(out=kTd[CR:128, :], in_=kTd[0:CR, :])

            # ---- phase 3: attention over 4 blocks of 512 query rows ----
            # Key tiles processed in pairs: two row-packed score matmuls,
            # one exp, then (pipelined one pair back) 8 o-matmuls + sums.
            with tc.tile_pool(name="ps3", bufs=1, space="PSUM") as p3:
                # prefetch all residual x row tiles (DMA is idle in phase 3)
                xr_tiles = []
                for rt in range(16):
                    xr = xres_pool.tile(
                        [128, C], f32, tag=f"xr{rt}", bufs=1, name=f"xr{rt}"
                    )
                    nc.sync.dma_start(
                        out=xr[:], in_=x_d[rt * 128 : (rt + 1) * 128, :]
                    )
                    xr_tiles.append(xr)
                for blk in range(4):
                    o_ps = [
                        p3.tile(
                            [128, C], f32, tag=f"o{rc}", name=f"ops{blk}_{rc}"
                        )
                        for rc in range(4)
                    ]
                    s_ps = p3.tile([128, 4], f32, tag="sums", name=f"sps{blk}")

                    def emit_o_pair(expair, ktbase):
                        # add the pair's two key tiles on VectorE so the
                        # row-sum needs only 4 ones-matmuls per pair, not 8
                        exs = ex_pool.tile(
                            [128, 512], bf16, tag="exs", bufs=2, name="exs"
                        )
                        nc.vector.tensor_add(
                            exs[:], expair[:, 0:512], expair[:, 512:1024]
                        )
                        for sub in range(2):
                            kt = ktbase + sub
                            for rc in range(4):
                                lhs = expair[
                                    :,
                                    sub * 512
                                    + rc * 128 : sub * 512
                                    + (rc + 1) * 128,
                                ]
                                nc.tensor.matmul(
                                    o_ps[rc][:],
                                    lhsT=lhs,
                                    rhs=v_t[kt][:],
                                    start=(kt == 0),
                                    stop=(kt == NKT - 1),
                                )
                                if sub == 1:
                                    nc.tensor.matmul(
                                        s_ps[:, rc : rc + 1],
                                        lhsT=exs[:, rc * 128 : (rc + 1) * 128],
                                        rhs=ones_sb[:],
                                        start=(ktbase == 0),
                                        stop=(ktbase == NKT - 2),
                                        skip_group_check=True,
                                    )

                    prev = None
                    for ktp in range(NKT // 2):
                        scp = p3.tile(
                            [128, 1024], f32, tag="sc", bufs=1, name="scp"
                        )
                        for sub in range(2):
                            kt = 2 * ktp + sub
                            hp = sub * CR
                            nc.tensor.matmul(
                                scp[:, sub * 512 : (sub + 1) * 512],
                                lhsT=kTd[
                                    hp : hp + CR, kt * 128 : (kt + 1) * 128
                                ],
                                rhs=qTd[
                                    hp : hp + CR,
                                    blk * 512 : (blk + 1) * 512,
                                ],
                                start=True,
                                stop=True,
                            )
                        expair = ex_pool.tile(
                            [128, 1024], bf16, tag="ex", name="ex"
                        )
                        nc.scalar.activation(expair[:], scp[:], Exp)
                        if prev is not None:
                            emit_o_pair(*prev)
                        prev = (expair, 2 * ktp)
                    emit_o_pair(*prev)

                    scls = []
                    for rc in range(4):
                        rcp = sm_pool.tile([128, 1], f32, tag="rcp", name="rcp")
                        nc.vector.reciprocal(rcp[:], s_ps[:, rc : rc + 1])
                        scl = sm_pool.tile([128, 1], f32, tag="scl", name="scl")
                        nc.vector.tensor_scalar_mul(scl[:], rcp[:], gam_sb[:])
                        scls.append(scl)
                    for rc in range(4):
                        rt = blk * 4 + rc
                        ot = out_pool.tile([128, C], f32, tag="ot", name="ot")
                        nc.vector.scalar_tensor_tensor(
                            out=ot[:],
                            in0=o_ps[rc][:],
                            scalar=scls[rc][:],
                            in1=xr_tiles[rt][:],
                            op0=mult,
                            op1=add,
                        )
                        nc.sync.dma_start(
                            out=out_d[rt * 128 : (rt + 1) * 128, :], in_=ot[:]
                        )

    nc.compile()
    return nc


def _get_nc():
    if "nc" not in _BUILt:
        _BUILt["nc"] = _build()
    return _BUILt["nc"]


def make_in_maps(x, Wf, Wg, Wh, gamma):
    x = np.asarray(x, dtype=np.float32)
    gv = np.full((128, 1), np.float32(np.asarray(gamma).reshape(-1)[0]))
    wf = np.ascontiguousarray(np.asarray(Wf, dtype=np.float32))
    wg = np.ascontiguousarray(np.asarray(Wg, dtype=np.float32))
    wh = np.ascontiguousarray(np.asarray(Wh, dtype=np.float32))
    in_maps = []
    for core in range(NCORES):
        b, h = divmod(core, 2)
        xb = x[b].reshape(N, C)
        own = xb[h * R : (h + 1) * R]
        other = xb[(1 - h) * R : (2 - h) * R]
        xp = np.ascontiguousarray(np.concatenate([own, other], axis=0))
        in_maps.append(
            {"x": xp, "wf": wf, "wg": wg, "wh": wh, "gammav": gv}
        )
    return in_maps


def gather_out(results, x):
    out = np.empty((B, N, C), dtype=np.float32)
    for core in range(NCORES):
        b, h = divmod(core, 2)
        out[b, h * R : (h + 1) * R] = results[core]["out"]
    return out.reshape(B, H, W, C)


def run(x, Wf, Wg, Wh, gamma, **spmd_kwargs):
    from concourse.bass_utils import run_bass_kernel_spmd

    nc = _get_nc()
    in_maps = make_in_maps(x, Wf, Wg, Wh, gamma)
    res = run_bass_kernel_spmd(
        nc, in_maps, core_ids=list(range(NCORES)), **spmd_kwargs
    )
    return gather_out(res.results, x), res


def kernel(x, Wf, Wg, Wh, gamma):
    out, _ = run(x, Wf, Wg, Wh, gamma)
    return out



# revision 12
# speedup vs baseline: 1.1198x; 1.0152x over previous
"""Trainium2 Bass kernel for nn_AttentionBlock (B=4, H=W=64, C=512, Cr=64).

Reference computation (per batch sample b):
    xf = x[b].reshape(N=4096, C=512)
    q = xf @ Wf; k = xf @ Wg; v = xf @ Wh
    attn = softmax(q @ k.T, axis=-1)
    out[b] = gamma * (attn @ v) + x[b]

Sharding: 8 cores, data-parallel over B=4 with 2-way sequence-parallel over
query rows. Core c handles batch c//2, query-row half c%2 (2048 rows).
Each core receives the full 4096x512 x of its batch, permuted so its OWN
2048 query rows come first (softmax over keys is permutation invariant as
long as k and v use the same key order, which they do). The program is
identical on all cores (SPMD); only the input data differs.

Per-core dataflow (matmuls bf16, f32 accumulation in PSUM):
  1. DMA x row tiles, cast bf16 on VectorE, PE-transpose -> xT [C, 4096].
  2. qT = Wf.T @ xT[:, :2048]; kT = Wg.T @ xT; v = x @ Wh (per-key-tile).
     kT/qT are duplicated onto partitions 64..127 (SBUF->SBUF DMA) so the
     K=64 score matmuls can row-pack onto disjoint halves of the PE array.
  3. Per 512-row block, key tiles processed in PAIRS: two back-to-back
     score matmuls scoresT[keys,rows] = kT_tile.T @ qT_block on array
     halves h0/h64 run concurrently; one exp over both on ScalarE (no max
     subtraction: |scores| < 60 is fp32/bf16-safe); o += exp.T @ v_tile
     accumulated over all 32 key tiles (software-pipelined one pair deep
     so PE never waits on exp); row sums via N=1 ones-matmuls.
  4. out = (o * gamma/sum) + x fused on VectorE, DMA out.
"""

import sys

if "/opt/trn_rl_repo" not in sys.path:
    sys.path.insert(0, "/opt/trn_rl_repo")

import numpy as np

_BUILt = {}

B, H, W, C = 4, 64, 64, 512
CR = 64          # C // reduction ratio
N = H * W        # 4096 keys per batch
R = N // 2       # 2048 query rows per core
NCORES = 8
NKT = N // 128   # 32 key tiles
KC = C // 128    # 4 contraction chunks over C


def _build():
    import ml_dtypes
    import concourse.bass as bass
    import concourse.mybir as mybir
    import concourse.tile as tile
    from concourse import bacc

    f32 = mybir.dt.float32
    bf16 = mybir.dt.bfloat16
    Exp = mybir.ActivationFunctionType.Exp
    mult = mybir.AluOpType.mult
    add = mybir.AluOpType.add

    nc = bacc.Bacc(
        "TRN2",
        target_bir_lowering=False,
        debug=False,
        num_devices=NCORES,
    )

    x_d = nc.dram_tensor("x", [N, C], f32, kind="ExternalInput")
    wf_d = nc.dram_tensor("wf", [C, CR], f32, kind="ExternalInput")
    wg_d = nc.dram_tensor("wg", [C, CR], f32, kind="ExternalInput")
    wh_d = nc.dram_tensor("wh", [C, C], f32, kind="ExternalInput")
    gam_d = nc.dram_tensor("gammav", [128, 1], f32, kind="ExternalInput")
    out_d = nc.dram_tensor("out", [R, C], f32, kind="ExternalOutput")

    identb_d = nc.inline_tensor(
        np.eye(128, dtype=ml_dtypes.bfloat16), name="identbc"
    )
    ones_d = nc.inline_tensor(
        np.ones((128, 1), dtype=ml_dtypes.bfloat16), name="onesc"
    )

    with tile.TileContext(nc) as tc:
        with (
            tc.tile_pool(name="const", bufs=1) as cpool,
            tc.tile_pool(name="stand", bufs=1) as spool,
            tc.tile_pool(name="xin", bufs=8) as xin_pool,
            tc.tile_pool(name="wtmp", bufs=2) as wtmp_pool,
            tc.tile_pool(name="exp", bufs=3) as ex_pool,
            tc.tile_pool(name="small", bufs=8) as sm_pool,
            tc.tile_pool(name="xres", bufs=4) as xres_pool,
            tc.tile_pool(name="outp", bufs=4) as out_pool,
        ):
            # ---- constants (only the transpose identity up front; the
            # rest rides behind the first x-row DMAs) ----
            identb_sb = cpool.tile([128, 128], bf16, name="identb_sb")
            nc.sync.dma_start(out=identb_sb[:], in_=identb_d[:])
            ones_sb = cpool.tile([128, 1], bf16, name="ones_sb")
            gam_sb = cpool.tile([128, 1], f32, name="gam_sb")

            wf_sb = cpool.tile([128, KC * CR], bf16, name="wf_sb")
            wg_sb = cpool.tile([128, KC * CR], bf16, name="wg_sb")
            wh_sb = cpool.tile([128, KC * C], bf16, name="wh_sb")

            def emit_weights():
                nc.sync.dma_start(out=ones_sb[:], in_=ones_d[:])
                nc.sync.dma_start(out=gam_sb[:], in_=gam_d[:])
                for w_d, w_sb, cols in (
                    (wg_d, wg_sb, CR),
                    (wh_d, wh_sb, C),
                    (wf_d, wf_sb, CR),
                ):
                    for kc in range(KC):
                        wt = wtmp_pool.tile(
                            [128, cols], f32, tag="wt", name="wt"
                        )
                        nc.sync.dma_start(
                            out=wt[:], in_=w_d[kc * 128 : (kc + 1) * 128, :]
                        )
                        nc.vector.tensor_copy(
                            w_sb[:, kc * cols : (kc + 1) * cols], wt[:]
                        )

            # ---- standing bf16 tensors ----
            xTa = [
                spool.tile([128, R], bf16, name=f"xTa{kc}") for kc in range(KC)
            ]
            xTb = [
                spool.tile([128, R], bf16, name=f"xTb{kc}") for kc in range(KC)
            ]
            v_t = [
                spool.tile([128, C], bf16, name=f"v{kt}") for kt in range(NKT)
            ]
            # kT/qT with a duplicate copy on partitions 64..127
            kTd = spool.tile([128, N], bf16, name="kTd")
            qTd = spool.tile([128, R], bf16, name="qTd")

            with (
                tc.tile_pool(name="ps12", bufs=3, space="PSUM") as tp_pool,
                tc.tile_pool(name="ps2kq", bufs=2, space="PSUM") as kq_pool,
                tc.tile_pool(name="ps2v", bufs=2, space="PSUM") as vps_pool,
            ):
                # ---- phase 1+2: transpose x; compute qT, kT, v ----
                def load_transpose_half(xT, half):
                    for rt16 in range(16):
                        rt = half * 16 + rt16
                        if rt == 8:
                            emit_weights()
                        xt = xin_pool.tile([128, C], f32, tag="xt", name="xt")
                        nc.sync.dma_start(
                            out=xt[:], in_=x_d[rt * 128 : (rt + 1) * 128, :]
                        )
                        xb = xin_pool.tile([128, C], bf16, tag="xb", name="xb")
                        nc.vector.tensor_copy(xb[:], xt[:])
                        tpt = tp_pool.tile(
                            [128, 512], bf16, tag="tp", name="tpt"
                        )
                        for kc in range(KC):
                            nc.tensor.transpose(
                                tpt[:, kc * 128 : (kc + 1) * 128],
                                xb[:, kc * 128 : (kc + 1) * 128],
                                identb_sb[:],
                            )
                        for kc in range(KC):
                            dst = xT[kc][:, rt16 * 128 : (rt16 + 1) * 128]
                            src = tpt[:, kc * 128 : (kc + 1) * 128]
                            if kc % 2 == 0:
                                nc.vector.tensor_copy(dst, src)
                            else:
                                nc.scalar.copy(dst, src)

                def emit_kq(w_sb, dst_sb, xT, nt_local, dst_off):
                    ps = kq_pool.tile([CR, 512], f32, tag="kq", name="kqp")
                    for kc in range(KC):
                        nc.tensor.matmul(
                            ps[:],
                            lhsT=w_sb[:, kc * CR : (kc + 1) * CR],
                            rhs=xT[kc][:, nt_local * 512 : (nt_local + 1) * 512],
                            start=(kc == 0),
                            stop=(kc == KC - 1),
                        )
                    nc.scalar.copy(dst_sb[0:CR, dst_off : dst_off + 512], ps[:])

                def emit_v(xT, kt):
                    kt16 = kt % 16
                    ps = vps_pool.tile([128, C], f32, tag="vps", name="vp")
                    for kc in range(KC):
                        nc.tensor.matmul(
                            ps[:],
                            lhsT=xT[kc][:, kt16 * 128 : (kt16 + 1) * 128],
                            rhs=wh_sb[:, kc * C : (kc + 1) * C],
                            start=(kc == 0),
                            stop=(kc == KC - 1),
                        )
                    nc.scalar.copy(v_t[kt][:], ps[:])

                load_transpose_half(xTa, 0)
                for nt in range(R // 512):
                    emit_kq(wf_sb, qTd, xTa, nt, nt * 512)
                # duplicate qT onto partitions 64..127
                nc.sync.dma_start(out=qTd[CR:128, :], in_=qTd[0:CR, :])
                for nt in range(4):
                    emit_kq(wg_sb, kTd, xTa, nt, nt * 512)
                for kt in range(16):
                    emit_v(xTa, kt)
                load_transpose_half(xTb, 1)
                for nt in range(4):
                    emit_kq(wg_sb, kTd, xTb, nt, 2048 + nt * 512)
                for kt in range(16, 32):
                    emit_v(xTb, kt)
                # duplicate kT onto partitions 64..127
                nc.sync.dma_start(out=kTd[CR:128, :], in_=kTd[0:CR, :])

            # ---- phase 3: attention over 4 blocks of 512 query rows ----
            # Key tiles processed in pairs: two row-packed score matmuls,
            # one exp, then (pipelined one pair back) 8 o-matmuls + sums.
            with tc.tile_pool(name="ps3", bufs=1, space="PSUM") as p3:
                # prefetch all residual x row tiles (DMA is idle in phase 3)
                xr_tiles = []
                for rt in range(16):
                    xr = xres_pool.tile(
                        [128, C], f32, tag=f"xr{rt}", bufs=1, name=f"xr{rt}"
                    )
                    nc.sync.dma_start(
                        out=xr[:], in_=x_d[rt * 128 : (rt + 1) * 128, :]
                    )
                    xr_tiles.append(xr)
                for blk in range(4):
                    o_ps = [
                        p3.tile(
                            [128, C], f32, tag=f"o{rc}", name=f"ops{blk}_{rc}"
                        )
                        for rc in range(4)
                    ]
                    s_ps = p3.tile([128, 4], f32, tag="sums", name=f"sps{blk}")

                    def emit_o_pair(expair, ktbase):
                        # add the pair's two key tiles on VectorE so the
                        # row-sum needs only 4 ones-matmuls per pair, not 8
                        exs = ex_pool.tile(
                            [128, 512], bf16, tag="exs", bufs=2, name="exs"
                        )
                        nc.vector.tensor_add(
                            exs[:], expair[:, 0:512], expair[:, 512:1024]
                        )
                        for sub in range(2):
                            kt = ktbase + sub
                            for rc in range(4):
                                lhs = expair[
                                    :,
                                    sub * 512
                                    + rc * 128 : sub * 512
                                    + (rc + 1) * 128,
                                ]
                                nc.tensor.matmul(
                                    o_ps[rc][:],
                                    lhsT=lhs,
                                    rhs=v_t[kt][:],
                                    start=(kt == 0),
                                    stop=(kt == NKT - 1),
                                )
                                if sub == 1:
                                    nc.tensor.matmul(
                                        s_ps[:, rc : rc + 1],
                                        lhsT=exs[:, rc * 128 : (rc + 1) * 128],
                                        rhs=ones_sb[:],
                                        start=(ktbase == 0),
                                        stop=(ktbase == NKT - 2),
                                        skip_group_check=True,
                                    )

                    prev = None
                    for ktp in range(NKT // 2):
                        # scores psum as two single-bank tiles in a 3-deep
                        # ring: pair p+1's first score matmul can start as
                        # soon as exp of pair p's first half is done, instead
                        # of serializing behind the whole previous exp
                        expair = ex_pool.tile(
                            [128, 1024], bf16, tag="ex", name="ex"
                        )
                        for sub in range(2):
                            kt = 2 * ktp + sub
                            hp = sub * CR
                            sch = p3.tile(
                                [128, 512], f32, tag="sc", bufs=3, name="sch"
                            )
                            nc.tensor.matmul(
                                sch[:],
                                lhsT=kTd[
                                    hp : hp + CR, kt * 128 : (kt + 1) * 128
                                ],
                                rhs=qTd[
                                    hp : hp + CR,
                                    blk * 512 : (blk + 1) * 512,
                                ],
                                start=True,
                                stop=True,
                            )
                            nc.scalar.activation(
                                expair[:, sub * 512 : (sub + 1) * 512],
                                sch[:], Exp,
                            )
                        if prev is not None:
                            emit_o_pair(*prev)
                        prev = (expair, 2 * ktp)
                    emit_o_pair(*prev)

                    scls = []
                    for rc in range(4):
                        rcp = sm_pool.tile([128, 1], f32, tag="rcp", name="rcp")
                        nc.vector.reciprocal(rcp[:], s_ps[:, rc : rc + 1])
                        scl = sm_pool.tile([128, 1], f32, tag="scl", name="scl")
                        nc.vector.tensor_scalar_mul(scl[:], rcp[:], gam_sb[:])
                        scls.append(scl)
                    for rc in range(4):
                        rt = blk * 4 + rc
                        ot = out_pool.tile([128, C], f32, tag="ot", name="ot")
                        nc.vector.scalar_tensor_tensor(
                            out=ot[:],
                            in0=o_ps[rc][:],
                            scalar=scls[rc][:],
                            in1=xr_tiles[rt][:],
                            op0=mult,
                            op1=add,
                        )
                        nc.sync.dma_start(
                            out=out_d[rt * 128 : (rt + 1) * 128, :], in_=ot[:]
                        )

    nc.compile()
    return nc


def _get_nc():
    if "nc" not in _BUILt:
        _BUILt["nc"] = _build()
    return _BUILt["nc"]


def make_in_maps(x, Wf, Wg, Wh, gamma):
    x = np.asarray(x, dtype=np.float32)
    gv = np.full((128, 1), np.float32(np.asarray(gamma).reshape(-1)[0]))
    wf = np.ascontiguousarray(np.asarray(Wf, dtype=np.float32))
    wg = np.ascontiguousarray(np.asarray(Wg, dtype=np.float32))
    wh = np.ascontiguousarray(np.asarray(Wh, dtype=np.float32))
    in_maps = []
    for core in range(NCORES):
        b, h = divmod(core, 2)
        xb = x[b].reshape(N, C)
        own = xb[h * R : (h + 1) * R]
        other = xb[(1 - h) * R : (2 - h) * R]
        xp = np.ascontiguousarray(np.concatenate([own, other], axis=0))
        in_maps.append(
            {"x": xp, "wf": wf, "wg": wg, "wh": wh, "gammav": gv}
        )
    return in_maps


def gather_out(results, x):
    out = np.empty((B, N, C), dtype=np.float32)
    for core in range(NCORES):
        b, h = divmod(core, 2)
        out[b, h * R : (h + 1) * R] = results[core]["out"]
    return out.reshape(B, H, W, C)


def run(x, Wf, Wg, Wh, gamma, **spmd_kwargs):
    from concourse.bass_utils import run_bass_kernel_spmd

    nc = _get_nc()
    in_maps = make_in_maps(x, Wf, Wg, Wh, gamma)
    res = run_bass_kernel_spmd(
        nc, in_maps, core_ids=list(range(NCORES)), **spmd_kwargs
    )
    return gather_out(res.results, x), res


def kernel(x, Wf, Wg, Wh, gamma):
    out, _ = run(x, Wf, Wg, Wh, gamma)
    return out

